# revision 15
# baseline (speedup 1.0000x reference)
"""Dense causal transformer attention block on 8 Trainium2 NeuronCores.

Problem: out = CausalAttention(RoPE(x@wq, x@wk), x@wv) @ wo
  x [2, 4096, 2048], 16 heads x 128 dim, fp32 I/O.

Sharding: tensor-parallel over heads. Core c owns heads {2c, 2c+1}:
  - computes qT/kT ([head_dim, seq] layout) for its heads from the
    host-packed xP (all DMAs are contiguous-row), RoPE applied on-chip in
    bf16 (one ScalarE PSUM->SBUF copy, then 2x-mode DVE ops),
  - V is projected directly in [seq, head_dim] layout by using the x tile
    as the matmul stationary operand (no PE transposes needed),
  - runs causal attention in transposed form (scoresT = k @ qT so the
    softmax weights come out as the moving operand of the A@V matmul),
    with partial-width A@V matmuls on the diagonal band,
  - denominators via an all-ones [128,128] stationary matmul on
    DVE-pre-summed exp tiles; reciprocal via the fast custom DVE op,
  - output projection for query tile t is deferred into tile t+1's
    attention phase (interleaved m-blocks) so the softmax-normalize chain
    never stalls the in-order PE queue.
Host sums the 8 partial outputs (the wo row-parallel all-reduce).

Compute dtype bf16 (PE 1 col/cycle), accumulation fp32 in PSUM.
"""
import sys

for _p in ("/opt/trn_rl_repo",):
    if _p not in sys.path:
        sys.path.insert(0, _p)

import numpy as np
import ml_dtypes
from contextlib import ExitStack

import concourse.bass as bass
import concourse.tile as tile
from concourse import bacc, mybir
from concourse import bass_utils

B, S, D = 2, 4096, 2048
H, DH = 16, 128
HALF = DH // 2
NC = 8
HPC = H // NC          # heads per core = 2
DOUT = HPC * DH        # 256 local proj width
ROPE_BASE = 10000.0
SCALE = 1.0 / float(np.sqrt(DH))
SQ = 512               # query tile (free dim of scoresT)
SKB = 128              # key block (partitions of scoresT)
KM = D // 128          # 16 contraction blocks
NSQ = S // SQ          # 8 query tiles per batch
BF = mybir.dt.bfloat16
F32 = mybir.dt.float32

_CACHED = {}


def _build():
    nc = bacc.Bacc("TRN2", target_bir_lowering=False, debug=False, num_devices=NC)

    # xP: [128, (hh, b*8+t, a*512+n)] so each (b,t,hh) x-tile DMA is a
    # contiguous [128, 4096] read. Weights pre-packed the same way.
    xP = nc.dram_tensor("xP", [128, 2 * B * NSQ * 8 * SQ], BF,
                        kind="ExternalInput").ap()
    wq = nc.dram_tensor("wq", [128, KM * DOUT], BF, kind="ExternalInput").ap()
    wk = nc.dram_tensor("wk", [128, KM * DOUT], BF, kind="ExternalInput").ap()
    wv = nc.dram_tensor("wv", [128, KM * DOUT], BF, kind="ExternalInput").ap()
    wo = nc.dram_tensor("wo", [128, HPC * D], BF, kind="ExternalInput").ap()
    cosf = nc.dram_tensor("cosf", [DH, S], BF, kind="ExternalInput").ap()
    sins = nc.dram_tensor("sins", [DH, S], BF, kind="ExternalInput").ap()
    masks = nc.dram_tensor("masks", [SKB, 4 * SQ], BF, kind="ExternalInput").ap()
    ones = nc.dram_tensor("ones", [128, 128], BF, kind="ExternalInput").ap()
    outp = nc.dram_tensor("outp", [B * S, D], BF, kind="ExternalOutput").ap()

    XTILE = 8 * SQ     # 4096 cols per (b,t,hh) x tile

    with tile.TileContext(nc) as tc, ExitStack() as ctx:
        const = ctx.enter_context(tc.tile_pool(name="const", bufs=1))
        xpool = ctx.enter_context(tc.tile_pool(name="xpool", bufs=1))
        qkv = ctx.enter_context(tc.tile_pool(name="qkv", bufs=1))
        rope = ctx.enter_context(tc.tile_pool(name="rope", bufs=2))
        attn = ctx.enter_context(tc.tile_pool(name="attn", bufs=4))
        opool = ctx.enter_context(tc.tile_pool(name="opool", bufs=2))

        # ---- persistent constants (DMAs emitted in priority order) -----
        # wq/wk split in halves (km 0-7 / 8-15) so the first projection
        # matmuls start as soon as ~0.5 MB has landed.
        wq_sb = [const.tile([128, KM * DOUT // 2], BF, name=f"wq_sb{i}")
                 for i in range(2)]
        wk_sb = [const.tile([128, KM * DOUT // 2], BF, name=f"wk_sb{i}")
                 for i in range(2)]
        wv_sb = const.tile([128, KM * DOUT], BF, name="wv_sb")
        ones_sb = const.tile([128, 128], BF, name="ones_sb")
        cos_sb = const.tile([DH, S], BF, name="cos_sb")
        sin_sb = const.tile([DH, S], BF, name="sin_sb")  # rows 64-127 = -sin
        mask_sb = const.tile([SKB, 4 * SQ], BF, name="mask_sb")
        wo_sb = const.tile([128, HPC * D], BF, name="wo_sb")   # [p, jj*2048+n]

        qT = [qkv.tile([128, S], BF, tag=f"qT{j}", name=f"qT{j}") for j in range(HPC)]
        kT = [qkv.tile([128, S], BF, tag=f"kT{j}", name=f"kT{j}") for j in range(HPC)]
        # vsb: [seq-block u][j*128+dh] packed, both heads interleaved
        vsb = qkv.tile([128, (S // 128) * DOUT], BF, tag="vsb", name="vsb")
        oT = [qkv.tile([128, S], BF, tag=f"oT{j}", name=f"oT{j}") for j in range(HPC)]

        with tc.tile_pool(name="psm", bufs=1, space="PSUM") as psm:
            # PSUM banks: pqk 2 (projections + deferred out-proj pf),
            # pscr 2x2 (attention score pairs), po 1, pd 1 = 8 banks.

            def emit_outproj_block(prev, mb, tags=("pqk",)):
                """Out-proj m-block mb (0..3) of the previous query tile.
                8 matmuls + 4 evacuations + 1 row DMA; interleaved into the
                attention phase to fill exp-paced PE gaps."""
                if prev is None:
                    return
                pb, pt = prev
                m = 4 * pt + mb
                ob = opool.tile([128, D], BF, tag="ob", bufs=2, name="ob")
                for n in range(D // 512):
                    pf = psm.tile([128, 512], F32, tag=tags[n % len(tags)],
                                  bufs=2, name="pf")
                    for jj in range(HPC):
                        nc.tensor.matmul(
                            pf[:], oT[jj][:, m * 128:(m + 1) * 128],
                            wo_sb[:, jj * D + n * 512: jj * D + (n + 1) * 512],
                            start=jj == 0, stop=jj == HPC - 1)
                    nc.vector.tensor_copy(ob[:, n * 512:(n + 1) * 512], pf[:])
                nc.sync.dma_start(
                    outp[pb * S + m * 128: pb * S + (m + 1) * 128, :], ob[:])

            prev = None
            for b in range(B):
                for t in range(NSQ):
                    s0 = t * SQ
                    bt = b * NSQ + t
                    # --- x tile: 4 quarter tiles, contiguous DMAs -------
                    # quarter q holds contraction blocks km = 4q..4q+3.
                    xbt = [xpool.tile([128, XTILE // 2], BF, tag="xb", bufs=8,
                                      name=f"xbt{qq}") for qq in range(4)]
                    xsrc = [xP[:, ((qq // 2) * B * NSQ + bt) * XTILE
                               + (qq % 2) * (XTILE // 2):
                               ((qq // 2) * B * NSQ + bt) * XTILE
                               + (qq % 2 + 1) * (XTILE // 2)]
                            for qq in range(4)]
                    if bt == 0:
                        # x quarters on the sync HW queue; weights/tables on
                        # the scalar HW queue so the transfers run in
                        # parallel and the first matmul starts ~9us in.
                        for qq in range(4):
                            nc.sync.dma_start(xbt[qq][:], xsrc[qq])
                        nc.scalar.dma_start(wq_sb[0][:], wq[:, 0:KM * DOUT // 2])
                        nc.scalar.dma_start(wq_sb[1][:], wq[:, KM * DOUT // 2:])
                        nc.scalar.dma_start(wk_sb[0][:], wk[:, 0:KM * DOUT // 2])
                        nc.scalar.dma_start(wk_sb[1][:], wk[:, KM * DOUT // 2:])
                        nc.scalar.dma_start(wv_sb[:], wv[:])
                        nc.scalar.dma_start(cos_sb[:], cosf[:])
                        nc.scalar.dma_start(sin_sb[:], sins[:])
                        nc.scalar.dma_start(mask_sb[:], masks[:])
                        nc.scalar.dma_start(wo_sb[:], wo[:])
                        nc.scalar.dma_start(ones_sb[:], ones[:])
                    else:
                        for qq in range(4):
                            nc.sync.dma_start(xbt[qq][:], xsrc[qq])

                    def xsl(km, c0, c1):
                        return xbt[km // 4][:, (km % 4) * SQ + c0:
                                            (km % 4) * SQ + c1]

                    # --- q/k projections + RoPE, head by head -----------
                    for j in range(HPC):
                        for w_sb, dstt in ((wq_sb, qT[j]), (wk_sb, kT[j])):
                            pp = psm.tile([128, SQ], F32, tag="pqk", bufs=2,
                                          name="pp")
                            for km in range(KM):
                                nc.tensor.matmul(
                                    pp[:],
                                    w_sb[km // 8][
                                        :, (km % 8) * DOUT + j * DH:
                                        (km % 8) * DOUT + (j + 1) * DH],
                                    xsl(km, 0, SQ),
                                    start=km == 0, stop=km == KM - 1)
                            ppb = rope.tile([128, SQ], BF, tag="ppb", bufs=3,
                                            name="ppb")
                            nc.scalar.copy(ppb[:], pp[:])
                            rt = rope.tile([128, SQ], BF, tag="rot", bufs=2,
                                           name="rt")
                            # sin_sb rows 0:64 = +sin, rows 64:128 = -sin so
                            # both SBUF inputs share a base partition.
                            nc.vector.tensor_mul(
                                rt[0:HALF, :], ppb[HALF:128, :],
                                sin_sb[HALF:128, s0:s0 + SQ])
                            nc.vector.tensor_mul(
                                rt[HALF:128, :], ppb[0:HALF, :],
                                sin_sb[0:HALF, s0:s0 + SQ])
                            m1 = rope.tile([128, SQ], BF, tag="m1", bufs=2,
                                           name="m1")
                            nc.vector.tensor_mul(m1[:], ppb[:],
                                                 cos_sb[:, s0:s0 + SQ])
                            nc.vector.tensor_add(dstt[:, s0:s0 + SQ],
                                                 m1[:], rt[:])

                    # --- V projection directly in [seq, dh] layout ------
                    # stationary = x tile slice, moving = wv -> out rows are
                    # sequence positions; no transpose needed. Two seq
                    # sub-blocks share one PSUM bank (two matmul groups).
                    for sbp in range(2):
                        pv = psm.tile([128, 2 * DOUT], F32, tag="pqk", bufs=2,
                                      name="pv")
                        for sh in range(2):
                            sb = 2 * sbp + sh
                            for km in range(KM):
                                nc.tensor.matmul(
                                    pv[:, sh * DOUT:(sh + 1) * DOUT],
                                    xsl(km, sb * 128, (sb + 1) * 128),
                                    wv_sb[:, km * DOUT:(km + 1) * DOUT],
                                    start=km == 0, stop=km == KM - 1,
                                    skip_group_check=True)
                        u = 4 * t + 2 * sbp
                        nc.scalar.copy(
                            vsb[:, u * DOUT:(u + 2) * DOUT], pv[:])

                    # --- causal attention for this query tile -----------
                    # All 4 deferred out-proj blocks are emitted inside the
                    # j=0 phase (before any oT write of this tile) so they
                    # never wait on this tile's normalize chain.
                    for j in range(HPC):
                        nblk = 4 * t + 4
                        npair = nblk // 2
                        po = psm.tile([128, SQ], F32, tag="po", name="po")
                        pd = psm.tile([128, SQ], F32, tag="pd", name="pd")
                        prev_et = None
                        qs2s = []
                        for p in range(npair):
                            pscr = psm.tile([128, 2 * SQ], F32, tag="pscr",
                                            bufs=2, name="pscr")
                            diag = 2 * p >= 4 * t
                            for h in range(2):
                                u = 2 * p + h
                                off = (u - 4 * t) * SKB if (diag and bt) else 0
                                nc.tensor.matmul(
                                    pscr[:, h * SQ + off:(h + 1) * SQ],
                                    kT[j][:, u * SKB:(u + 1) * SKB],
                                    qT[j][:, s0 + off:s0 + SQ],
                                    start=True, stop=True,
                                    skip_group_check=True)
                            et = attn.tile([128, 2 * SQ], BF, tag="et", bufs=6,
                                           name="et")
                            if diag and bt and 2 * p - 4 * t == 2:
                                # steep diagonal pair: exp only the live
                                # regions [256:512] and [896:1024]
                                nc.scalar.activation(
                                    et[:, 256:512], pscr[:, 256:512],
                                    mybir.ActivationFunctionType.Exp,
                                    scale=SCALE)
                                nc.scalar.activation(
                                    et[:, 896:1024], pscr[:, 896:1024],
                                    mybir.ActivationFunctionType.Exp,
                                    scale=SCALE)
                            else:
                                nc.scalar.activation(
                                    et[:], pscr[:],
                                    mybir.ActivationFunctionType.Exp,
                                    scale=SCALE)
                            if diag:  # mask also zeroes any stale region
                                r = 2 * p - 4 * t   # 0 or 2
                                nc.vector.tensor_mul(
                                    et[:], et[:],
                                    mask_sb[:, r * SQ:(r + 2) * SQ])
                            for h in range(2):
                                u = 2 * p + h
                                off = (u - 4 * t) * SKB if diag else 0
                                nc.tensor.matmul(
                                    po[:, off:SQ],
                                    vsb[:, u * DOUT + j * DH:
                                        u * DOUT + (j + 1) * DH],
                                    et[:, h * SQ + off:(h + 1) * SQ],
                                    start=u == 0, stop=u == nblk - 1,
                                    skip_group_check=True)
                            if p % 2 == 1:
                                # pair-sum on GpSimd (otherwise idle) to
                                # keep DVE off the exp-paced critical path
                                qs = attn.tile([128, 2 * SQ], BF, tag="qs",
                                               bufs=3, name="qs")
                                nc.gpsimd.tensor_add(qs[:], prev_et[:], et[:])
                                qs2 = attn.tile([128, SQ], BF, tag="qs2",
                                                bufs=8, name="qs2")
                                nc.vector.tensor_add(
                                    qs2[:], qs[:, 0:SQ], qs[:, SQ:2 * SQ])
                                qs2s.append(qs2)
                            prev_et = et
                            # fill exp-paced gaps with deferred out-proj
                            if j == 0 and p in (0, 1):
                                emit_outproj_block(prev, p)
                        # pre-sum qs2 pairs on DVE to halve the ones-matmuls
                        dsum = []
                        for qi in range(0, len(qs2s) - 1, 2):
                            q4 = attn.tile([128, SQ], BF, tag="q4", bufs=4,
                                           name="q4")
                            nc.vector.tensor_add(
                                q4[:], qs2s[qi][:], qs2s[qi + 1][:])
                            dsum.append(q4)
                        if len(qs2s) % 2:
                            dsum.append(qs2s[-1])
                        for qi, q2 in enumerate(dsum):
                            nc.tensor.matmul(
                                pd[:], ones_sb[:], q2[:],
                                start=qi == 0, stop=qi == len(dsum) - 1)
                        if j == 0:
                            emit_outproj_block(prev, 2)
                        rec = attn.tile([128, SQ], F32, tag="rec", bufs=2,
                                        name="rec")
                        nc.vector.reciprocal_approx_fast(rec[:], pd[:])
                        if j == 0:
                            emit_outproj_block(prev, 3)
                        nc.vector.tensor_mul(oT[j][:, s0:s0 + SQ], po[:], rec[:])
                    prev = (b, t)
            # final tile's out-proj: alternate PSUM tags for deeper pipeline
            for mb in range(4):
                emit_outproj_block(prev, mb, tags=("pqk", "pscr"))

    nc.compile()
    return nc


def _host_inputs(x, wq, wk, wv, wo, cos, sin):
    bf16 = ml_dtypes.bfloat16
    # xP[p, hh, bt, a, n] = x[b, t*512+n, hh*1024 + a*128 + p]
    xb = np.ascontiguousarray(
        x.reshape(B * S, D).T).astype(bf16)           # [D, B*S]
    xP = np.ascontiguousarray(
        xb.reshape(2, 8, 128, B, NSQ, SQ)
        .transpose(2, 0, 3, 4, 1, 5).reshape(128, -1))

    def pack_w(w):  # [D, 256] -> [128, km*256+n]
        return np.ascontiguousarray(
            w.reshape(KM, 128, DOUT).transpose(1, 0, 2).reshape(128, -1)
        ).astype(bf16)

    cos = np.asarray(cos, dtype=np.float32)        # [S, 64]
    sin = np.asarray(sin, dtype=np.float32)
    cosf = np.ascontiguousarray(
        np.concatenate([cos, cos], axis=1).T).astype(bf16)   # [128, S]
    sinf = np.concatenate([sin, -sin], axis=1).T   # rows 64-127 negated
    sinf = np.ascontiguousarray(sinf).astype(bf16)

    i = np.arange(SKB)[:, None]
    jj = np.arange(SQ)[None, :]
    masks = np.concatenate(
        [(i + r * SKB <= jj) for r in range(4)], axis=1).astype(bf16)
    ones_h = np.ones((128, 128), dtype=bf16)

    in_maps = []
    for c in range(NC):
        lo = c * DOUT
        wop = np.ascontiguousarray(
            wo[lo:lo + DOUT, :].reshape(HPC, 128, D)
            .transpose(1, 0, 2).reshape(128, -1)).astype(bf16)
        in_maps.append({
            "xP": xP,
            "wq": pack_w(np.ascontiguousarray(wq[:, lo:lo + DOUT])),
            "wk": pack_w(np.ascontiguousarray(wk[:, lo:lo + DOUT])),
            "wv": pack_w(np.ascontiguousarray(wv[:, lo:lo + DOUT])),
            "wo": wop,
            "cosf": cosf,
            "sins": sinf,
            "masks": masks,
            "ones": ones_h,
        })
    return in_maps


def kernel(x, wq, wk, wv, wo, cos, sin, _trace=False, _tmpdir=None):
    if "nc" not in _CACHED:
        _CACHED["nc"] = _build()
    nc = _CACHED["nc"]
    in_maps = _host_inputs(
        np.asarray(x, dtype=np.float32), np.asarray(wq, dtype=np.float32),
        np.asarray(wk, dtype=np.float32), np.asarray(wv, dtype=np.float32),
        np.asarray(wo, dtype=np.float32), cos, sin)
    res = bass_utils.run_bass_kernel_spmd(
        nc, in_maps, core_ids=list(range(NC)), trace=_trace, tmpdir=_tmpdir)
    acc = np.zeros((B * S, D), dtype=np.float32)
    for c in range(NC):
        acc += res.results[c]["outp"].astype(np.float32)
    out = acc.reshape(B, S, D)
    if _trace:
        _CACHED["last_results"] = res
    return out


# revision 19
# speedup vs baseline: 1.1225x; 1.1225x over previous
"""Dense causal transformer attention block on 8 Trainium2 NeuronCores.

Problem: out = CausalAttention(RoPE(x@wq, x@wk), x@wv) @ wo
  x [2, 4096, 2048], 16 heads x 128 dim, fp32 I/O.

Sharding: tensor-parallel over heads. Core c owns heads {2c, 2c+1}:
  - computes qT/kT ([head_dim, seq] layout) for its heads from the
    host-packed xP (all DMAs are contiguous-row), RoPE applied on-chip in
    bf16 (one ScalarE PSUM->SBUF copy, then 2x-mode DVE ops),
  - V is projected directly in [seq, head_dim] layout by using the x tile
    as the matmul stationary operand (no PE transposes needed),
  - runs causal attention in transposed form (scoresT = k @ qT so the
    softmax weights come out as the moving operand of the A@V matmul),
    with partial-width A@V matmuls on the diagonal band,
  - denominators via an all-ones [128,128] stationary matmul on
    DVE-pre-summed exp tiles; reciprocal via the fast custom DVE op,
  - output projection for query tile t is deferred into tile t+1's
    attention phase (interleaved m-blocks) so the softmax-normalize chain
    never stalls the in-order PE queue.
Host sums the 8 partial outputs (the wo row-parallel all-reduce).

Compute dtype bf16 (PE 1 col/cycle), accumulation fp32 in PSUM.
"""
import sys

for _p in ("/opt/trn_rl_repo",):
    if _p not in sys.path:
        sys.path.insert(0, _p)

import numpy as np
import ml_dtypes
from contextlib import ExitStack

import concourse.bass as bass
import concourse.tile as tile
from concourse import bacc, mybir
from concourse import bass_utils

B, S, D = 2, 4096, 2048
H, DH = 16, 128
HALF = DH // 2
NC = 8
HPC = H // NC          # heads per core = 2
DOUT = HPC * DH        # 256 local proj width
ROPE_BASE = 10000.0
SCALE = 1.0 / float(np.sqrt(DH))
SQ = 512               # query tile (free dim of scoresT)
SKB = 128              # key block (partitions of scoresT)
KM = D // 128          # 16 contraction blocks
NSQ = S // SQ          # 8 query tiles per batch
BF = mybir.dt.bfloat16
F32 = mybir.dt.float32

_CACHED = {}


def _build():
    nc = bacc.Bacc("TRN2", target_bir_lowering=False, debug=False, num_devices=NC)

    # xP: [128, (hh, b*8+t, a*512+n)] so each (b,t,hh) x-tile DMA is a
    # contiguous [128, 4096] read. Weights pre-packed the same way.
    xP = nc.dram_tensor("xP", [128, 2 * B * NSQ * 8 * SQ], BF,
                        kind="ExternalInput").ap()
    wq = nc.dram_tensor("wq", [128, KM * DOUT], BF, kind="ExternalInput").ap()
    wk = nc.dram_tensor("wk", [128, KM * DOUT], BF, kind="ExternalInput").ap()
    wv = nc.dram_tensor("wv", [128, KM * DOUT], BF, kind="ExternalInput").ap()
    wo = nc.dram_tensor("wo", [128, HPC * D], BF, kind="ExternalInput").ap()
    cosf = nc.dram_tensor("cosf", [DH, S], BF, kind="ExternalInput").ap()
    sins = nc.dram_tensor("sins", [DH, S], BF, kind="ExternalInput").ap()
    masks = nc.dram_tensor("masks", [SKB, 4 * SQ], BF, kind="ExternalInput").ap()
    ones = nc.dram_tensor("ones", [128, 128], BF, kind="ExternalInput").ap()
    outp = nc.dram_tensor("outp", [B * S, D], BF, kind="ExternalOutput").ap()

    XTILE = 8 * SQ     # 4096 cols per (b,t,hh) x tile

    with tile.TileContext(nc) as tc, ExitStack() as ctx:
        const = ctx.enter_context(tc.tile_pool(name="const", bufs=1))
        xpool = ctx.enter_context(tc.tile_pool(name="xpool", bufs=1))
        qkv = ctx.enter_context(tc.tile_pool(name="qkv", bufs=1))
        rope = ctx.enter_context(tc.tile_pool(name="rope", bufs=2))
        attn = ctx.enter_context(tc.tile_pool(name="attn", bufs=4))
        opool = ctx.enter_context(tc.tile_pool(name="opool", bufs=2))

        # ---- persistent constants (DMAs emitted in priority order) -----
        # wq/wk split in halves (km 0-7 / 8-15) so the first projection
        # matmuls start as soon as ~0.5 MB has landed.
        wq_sb = [const.tile([128, KM * DOUT // 2], BF, name=f"wq_sb{i}")
                 for i in range(2)]
        wk_sb = [const.tile([128, KM * DOUT // 2], BF, name=f"wk_sb{i}")
                 for i in range(2)]
        wv_sb = const.tile([128, KM * DOUT], BF, name="wv_sb")
        ones_sb = const.tile([128, 128], BF, name="ones_sb")
        cos_sb = const.tile([DH, S], BF, name="cos_sb")
        sin_sb = const.tile([DH, S], BF, name="sin_sb")  # rows 64-127 = -sin
        mask_sb = const.tile([SKB, 4 * SQ], BF, name="mask_sb")
        wo_sb = const.tile([128, HPC * D], BF, name="wo_sb")   # [p, jj*2048+n]

        qT = [qkv.tile([128, S], BF, tag=f"qT{j}", name=f"qT{j}") for j in range(HPC)]
        kT = [qkv.tile([128, S], BF, tag=f"kT{j}", name=f"kT{j}") for j in range(HPC)]
        # vsb: [seq-block u][j*128+dh] packed, both heads interleaved
        vsb = qkv.tile([128, (S // 128) * DOUT], BF, tag="vsb", name="vsb")
        oT = [qkv.tile([128, S], BF, tag=f"oT{j}", name=f"oT{j}") for j in range(HPC)]

        with tc.tile_pool(name="psm", bufs=1, space="PSUM") as psm:
            # PSUM banks: pqk 2 (projections + deferred out-proj pf),
            # pscr 2x2 (attention score pairs), po 1, pd 1 = 8 banks.

            def emit_outproj_block(prev, mb, tags=("pqk",)):
                """Out-proj m-block mb (0..3) of the previous query tile.
                8 matmuls + 4 evacuations + 1 row DMA; interleaved into the
                attention phase to fill exp-paced PE gaps."""
                if prev is None:
                    return
                pb, pt = prev
                m = 4 * pt + mb
                ob = opool.tile([128, D], BF, tag="ob", bufs=2, name="ob")
                for n in range(D // 512):
                    pf = psm.tile([128, 512], F32, tag=tags[n % len(tags)],
                                  bufs=2, name="pf")
                    for jj in range(HPC):
                        nc.tensor.matmul(
                            pf[:], oT[jj][:, m * 128:(m + 1) * 128],
                            wo_sb[:, jj * D + n * 512: jj * D + (n + 1) * 512],
                            start=jj == 0, stop=jj == HPC - 1)
                    nc.vector.tensor_copy(ob[:, n * 512:(n + 1) * 512], pf[:])
                nc.sync.dma_start(
                    outp[pb * S + m * 128: pb * S + (m + 1) * 128, :], ob[:])

            prev = None
            for b in range(B):
                for t in range(NSQ):
                    s0 = t * SQ
                    bt = b * NSQ + t
                    # --- x tile: 4 quarter tiles, contiguous DMAs -------
                    # quarter q holds contraction blocks km = 4q..4q+3.
                    xbt = [xpool.tile([128, XTILE // 2], BF, tag="xb", bufs=8,
                                      name=f"xbt{qq}") for qq in range(4)]
                    xsrc = [xP[:, ((qq // 2) * B * NSQ + bt) * XTILE
                               + (qq % 2) * (XTILE // 2):
                               ((qq // 2) * B * NSQ + bt) * XTILE
                               + (qq % 2 + 1) * (XTILE // 2)]
                            for qq in range(4)]
                    if bt == 0:
                        # x quarters on the sync HW queue; weights/tables on
                        # the scalar HW queue so the transfers run in
                        # parallel and the first matmul starts ~9us in.
                        for qq in range(4):
                            nc.sync.dma_start(xbt[qq][:], xsrc[qq])
                        nc.scalar.dma_start(wq_sb[0][:], wq[:, 0:KM * DOUT // 2])
                        nc.scalar.dma_start(wq_sb[1][:], wq[:, KM * DOUT // 2:])
                        nc.scalar.dma_start(wk_sb[0][:], wk[:, 0:KM * DOUT // 2])
                        nc.scalar.dma_start(wk_sb[1][:], wk[:, KM * DOUT // 2:])
                        nc.scalar.dma_start(wv_sb[:], wv[:])
                        nc.scalar.dma_start(cos_sb[:], cosf[:])
                        nc.scalar.dma_start(sin_sb[:], sins[:])
                        nc.scalar.dma_start(mask_sb[:], masks[:])
                        nc.scalar.dma_start(wo_sb[:], wo[:])
                        nc.scalar.dma_start(ones_sb[:], ones[:])
                    else:
                        for qq in range(4):
                            nc.sync.dma_start(xbt[qq][:], xsrc[qq])

                    def xsl(km, c0, c1):
                        return xbt[km // 4][:, (km % 4) * SQ + c0:
                                            (km % 4) * SQ + c1]

                    # --- q/k projections + RoPE, head by head -----------
                    for j in range(HPC):
                        for w_sb, dstt in ((wq_sb, qT[j]), (wk_sb, kT[j])):
                            pp = psm.tile([128, SQ], F32, tag="pqk", bufs=2,
                                          name="pp")
                            for km in range(KM):
                                nc.tensor.matmul(
                                    pp[:],
                                    w_sb[km // 8][
                                        :, (km % 8) * DOUT + j * DH:
                                        (km % 8) * DOUT + (j + 1) * DH],
                                    xsl(km, 0, SQ),
                                    start=km == 0, stop=km == KM - 1)
                            ppb = rope.tile([128, SQ], BF, tag="ppb", bufs=3,
                                            name="ppb")
                            nc.scalar.copy(ppb[:], pp[:])
                            rt = rope.tile([128, SQ], BF, tag="rot", bufs=2,
                                           name="rt")
                            # sin_sb rows 0:64 = +sin, rows 64:128 = -sin so
                            # both SBUF inputs share a base partition.
                            nc.vector.tensor_mul(
                                rt[0:HALF, :], ppb[HALF:128, :],
                                sin_sb[HALF:128, s0:s0 + SQ])
                            nc.vector.tensor_mul(
                                rt[HALF:128, :], ppb[0:HALF, :],
                                sin_sb[0:HALF, s0:s0 + SQ])
                            m1 = rope.tile([128, SQ], BF, tag="m1", bufs=2,
                                           name="m1")
                            nc.vector.tensor_mul(m1[:], ppb[:],
                                                 cos_sb[:, s0:s0 + SQ])
                            nc.vector.tensor_add(dstt[:, s0:s0 + SQ],
                                                 m1[:], rt[:])

                    # --- V projection directly in [seq, dh] layout ------
                    # stationary = x tile slice, moving = wv -> out rows are
                    # sequence positions; no transpose needed. Two seq
                    # sub-blocks share one PSUM bank (two matmul groups).
                    for sbp in range(2):
                        pv = psm.tile([128, 2 * DOUT], F32, tag="pqk", bufs=2,
                                      name="pv")
                        for sh in range(2):
                            sb = 2 * sbp + sh
                            for km in range(KM):
                                nc.tensor.matmul(
                                    pv[:, sh * DOUT:(sh + 1) * DOUT],
                                    xsl(km, sb * 128, (sb + 1) * 128),
                                    wv_sb[:, km * DOUT:(km + 1) * DOUT],
                                    start=km == 0, stop=km == KM - 1,
                                    skip_group_check=True)
                        u = 4 * t + 2 * sbp
                        nc.scalar.copy(
                            vsb[:, u * DOUT:(u + 2) * DOUT], pv[:])

                    # --- causal attention for this query tile -----------
                    # Out-proj blocks mb0/mb1 fill the j=0 exp-paced gaps,
                    # mb2/mb3 fill j=1; the j=0 normalize-multiply is
                    # delayed past mb3 so the mb oT reads never wait on this
                    # tile's normalize chain (po is double-buffered).
                    delayed_mul = None
                    for j in range(HPC):
                        nblk = 4 * t + 4
                        npair = nblk // 2
                        po = psm.tile([128, SQ], F32, tag="po", bufs=2,
                                      name="po")
                        prev_et = None
                        qs2s = []
                        for p in range(npair):
                            pscr = psm.tile([128, 2 * SQ], F32, tag="pscr",
                                            bufs=2, name="pscr")
                            diag = 2 * p >= 4 * t
                            for h in range(2):
                                u = 2 * p + h
                                off = (u - 4 * t) * SKB if (diag and bt) else 0
                                nc.tensor.matmul(
                                    pscr[:, h * SQ + off:(h + 1) * SQ],
                                    kT[j][:, u * SKB:(u + 1) * SKB],
                                    qT[j][:, s0 + off:s0 + SQ],
                                    start=True, stop=True,
                                    skip_group_check=True)
                            et = attn.tile([128, 2 * SQ], BF, tag="et", bufs=6,
                                           name="et")
                            if diag and bt and 2 * p - 4 * t == 2:
                                # steep diagonal pair: exp only the live
                                # regions [256:512] and [896:1024]
                                nc.scalar.activation(
                                    et[:, 256:512], pscr[:, 256:512],
                                    mybir.ActivationFunctionType.Exp,
                                    scale=SCALE)
                                nc.scalar.activation(
                                    et[:, 896:1024], pscr[:, 896:1024],
                                    mybir.ActivationFunctionType.Exp,
                                    scale=SCALE)
                            else:
                                nc.scalar.activation(
                                    et[:], pscr[:],
                                    mybir.ActivationFunctionType.Exp,
                                    scale=SCALE)
                            if diag:  # mask also zeroes any stale region
                                r = 2 * p - 4 * t   # 0 or 2
                                nc.vector.tensor_mul(
                                    et[:], et[:],
                                    mask_sb[:, r * SQ:(r + 2) * SQ])
                            for h in range(2):
                                u = 2 * p + h
                                off = (u - 4 * t) * SKB if diag else 0
                                nc.tensor.matmul(
                                    po[:, off:SQ],
                                    vsb[:, u * DOUT + j * DH:
                                        u * DOUT + (j + 1) * DH],
                                    et[:, h * SQ + off:(h + 1) * SQ],
                                    start=u == 0, stop=u == nblk - 1,
                                    skip_group_check=True)
                            if p % 2 == 1:
                                qs = attn.tile([128, 2 * SQ], BF, tag="qs",
                                               bufs=3, name="qs")
                                nc.vector.tensor_add(qs[:], prev_et[:], et[:])
                                qs2 = attn.tile([128, SQ], BF, tag="qs2",
                                                bufs=8, name="qs2")
                                nc.vector.tensor_add(
                                    qs2[:], qs[:, 0:SQ], qs[:, SQ:2 * SQ])
                                qs2s.append(qs2)
                            prev_et = et
                            # fill exp-paced gaps with deferred out-proj
                            if p in (0, 1) and t > 0:
                                emit_outproj_block(prev, 2 * j + p)
                        # pre-sum qs2 pairs on DVE to halve the ones-matmuls
                        dsum = []
                        for qi in range(0, len(qs2s) - 1, 2):
                            q4 = attn.tile([128, SQ], BF, tag="q4", bufs=4,
                                           name="q4")
                            nc.vector.tensor_add(
                                q4[:], qs2s[qi][:], qs2s[qi + 1][:])
                            dsum.append(q4)
                        if len(qs2s) % 2:
                            dsum.append(qs2s[-1])
                        # pd lives briefly in the pqk rotation (alloc at use)
                        pd = psm.tile([128, SQ], F32, tag="pqk", bufs=2,
                                      name="pd")
                        for qi, q2 in enumerate(dsum):
                            nc.tensor.matmul(
                                pd[:], ones_sb[:], q2[:],
                                start=qi == 0, stop=qi == len(dsum) - 1)
                        if t == 0:
                            emit_outproj_block(prev, 2 * j)
                        rec = attn.tile([128, SQ], F32, tag="rec", bufs=2,
                                        name="rec")
                        nc.vector.reciprocal_approx_fast(rec[:], pd[:])
                        if t == 0:
                            emit_outproj_block(prev, 2 * j + 1)
                        if j == 0:
                            delayed_mul = (po, rec, s0)
                        else:
                            # j0's normalize-mul, delayed past mb2/mb3
                            dpo, drec, ds0 = delayed_mul
                            nc.vector.tensor_mul(
                                oT[0][:, ds0:ds0 + SQ], dpo[:], drec[:])
                            nc.vector.tensor_mul(
                                oT[1][:, s0:s0 + SQ], po[:], rec[:])
                    prev = (b, t)
            # final tile's out-proj: alternate PSUM tags for deeper pipeline
            for mb in range(4):
                emit_outproj_block(prev, mb, tags=("pqk", "pscr"))

    nc.compile()
    return nc


def _host_inputs(x, wq, wk, wv, wo, cos, sin):
    bf16 = ml_dtypes.bfloat16
    # xP[p, hh, bt, a, n] = x[b, t*512+n, hh*1024 + a*128 + p]
    xb = np.ascontiguousarray(
        x.reshape(B * S, D).T).astype(bf16)           # [D, B*S]
    xP = np.ascontiguousarray(
        xb.reshape(2, 8, 128, B, NSQ, SQ)
        .transpose(2, 0, 3, 4, 1, 5).reshape(128, -1))

    def pack_w(w):  # [D, 256] -> [128, km*256+n]
        return np.ascontiguousarray(
            w.reshape(KM, 128, DOUT).transpose(1, 0, 2).reshape(128, -1)
        ).astype(bf16)

    cos = np.asarray(cos, dtype=np.float32)        # [S, 64]
    sin = np.asarray(sin, dtype=np.float32)
    cosf = np.ascontiguousarray(
        np.concatenate([cos, cos], axis=1).T).astype(bf16)   # [128, S]
    sinf = np.concatenate([sin, -sin], axis=1).T   # rows 64-127 negated
    sinf = np.ascontiguousarray(sinf).astype(bf16)

    i = np.arange(SKB)[:, None]
    jj = np.arange(SQ)[None, :]
    masks = np.concatenate(
        [(i + r * SKB <= jj) for r in range(4)], axis=1).astype(bf16)
    ones_h = np.ones((128, 128), dtype=bf16)

    in_maps = []
    for c in range(NC):
        lo = c * DOUT
        wop = np.ascontiguousarray(
            wo[lo:lo + DOUT, :].reshape(HPC, 128, D)
            .transpose(1, 0, 2).reshape(128, -1)).astype(bf16)
        in_maps.append({
            "xP": xP,
            "wq": pack_w(np.ascontiguousarray(wq[:, lo:lo + DOUT])),
            "wk": pack_w(np.ascontiguousarray(wk[:, lo:lo + DOUT])),
            "wv": pack_w(np.ascontiguousarray(wv[:, lo:lo + DOUT])),
            "wo": wop,
            "cosf": cosf,
            "sins": sinf,
            "masks": masks,
            "ones": ones_h,
        })
    return in_maps


def kernel(x, wq, wk, wv, wo, cos, sin, _trace=False, _tmpdir=None):
    if "nc" not in _CACHED:
        _CACHED["nc"] = _build()
    nc = _CACHED["nc"]
    in_maps = _host_inputs(
        np.asarray(x, dtype=np.float32), np.asarray(wq, dtype=np.float32),
        np.asarray(wk, dtype=np.float32), np.asarray(wv, dtype=np.float32),
        np.asarray(wo, dtype=np.float32), cos, sin)
    res = bass_utils.run_bass_kernel_spmd(
        nc, in_maps, core_ids=list(range(NC)), trace=_trace, tmpdir=_tmpdir)
    acc = np.zeros((B * S, D), dtype=np.float32)
    for c in range(NC):
        acc += res.results[c]["outp"].astype(np.float32)
    out = acc.reshape(B, S, D)
    if _trace:
        _CACHED["last_results"] = res
    return out


# revision 24
# speedup vs baseline: 1.1517x; 1.0260x over previous
"""Dense causal transformer attention block on 8 Trainium2 NeuronCores.

Problem: out = CausalAttention(RoPE(x@wq, x@wk), x@wv) @ wo
  x [2, 4096, 2048], 16 heads x 128 dim, fp32 I/O.

Sharding: tensor-parallel over heads. Core c owns heads {2c, 2c+1}:
  - computes qT/kT ([head_dim, seq] layout) for its heads from the
    host-packed xP (all DMAs are contiguous-row), RoPE applied on-chip in
    bf16 (one ScalarE PSUM->SBUF copy, then 2x-mode DVE ops),
  - V is projected directly in [seq, head_dim] layout by using the x tile
    as the matmul stationary operand (no PE transposes needed),
  - runs causal attention in transposed form (scoresT = k @ qT so the
    softmax weights come out as the moving operand of the A@V matmul),
    with partial-width A@V matmuls on the diagonal band,
  - denominators via an all-ones [128,128] stationary matmul on
    DVE-pre-summed exp tiles; reciprocal via the fast custom DVE op,
  - output projection for query tile t is deferred into tile t+1's
    attention phase (interleaved m-blocks) so the softmax-normalize chain
    never stalls the in-order PE queue.
Host sums the 8 partial outputs (the wo row-parallel all-reduce).

Compute dtype bf16 (PE 1 col/cycle), accumulation fp32 in PSUM.
"""
import sys

for _p in ("/opt/trn_rl_repo",):
    if _p not in sys.path:
        sys.path.insert(0, _p)

import numpy as np
import ml_dtypes
from contextlib import ExitStack

import concourse.bass as bass
import concourse.tile as tile
from concourse import bacc, mybir
from concourse import bass_utils

B, S, D = 2, 4096, 2048
H, DH = 16, 128
HALF = DH // 2
NC = 8
HPC = H // NC          # heads per core = 2
DOUT = HPC * DH        # 256 local proj width
ROPE_BASE = 10000.0
SCALE = 1.0 / float(np.sqrt(DH))
SQ = 512               # query tile (free dim of scoresT)
SKB = 128              # key block (partitions of scoresT)
KM = D // 128          # 16 contraction blocks
NSQ = S // SQ          # 8 query tiles per batch
BF = mybir.dt.bfloat16
F32 = mybir.dt.float32

_CACHED = {}


def _build():
    nc = bacc.Bacc("TRN2", target_bir_lowering=False, debug=False, num_devices=NC)

    # xP: [128, (hh, b*8+t, a*512+n)] so each (b,t,hh) x-tile DMA is a
    # contiguous [128, 4096] read. Weights pre-packed the same way.
    xP = nc.dram_tensor("xP", [128, 2 * B * NSQ * 8 * SQ], BF,
                        kind="ExternalInput").ap()
    wq = nc.dram_tensor("wq", [128, KM * DOUT], BF, kind="ExternalInput").ap()
    wk = nc.dram_tensor("wk", [128, KM * DOUT], BF, kind="ExternalInput").ap()
    wv = nc.dram_tensor("wv", [128, KM * DOUT], BF, kind="ExternalInput").ap()
    wo = nc.dram_tensor("wo", [128, HPC * D], BF, kind="ExternalInput").ap()
    cosf = nc.dram_tensor("cosf", [DH, S], BF, kind="ExternalInput").ap()
    sins = nc.dram_tensor("sins", [DH, S], BF, kind="ExternalInput").ap()
    masks = nc.dram_tensor("masks", [SKB, 4 * SQ], BF, kind="ExternalInput").ap()
    ones = nc.dram_tensor("ones", [128, 128], BF, kind="ExternalInput").ap()
    outp = nc.dram_tensor("outp", [B * S, D], BF, kind="ExternalOutput").ap()

    XTILE = 8 * SQ     # 4096 cols per (b,t,hh) x tile

    with tile.TileContext(nc) as tc, ExitStack() as ctx:
        const = ctx.enter_context(tc.tile_pool(name="const", bufs=1))
        xpool = ctx.enter_context(tc.tile_pool(name="xpool", bufs=1))
        qkv = ctx.enter_context(tc.tile_pool(name="qkv", bufs=1))
        rope = ctx.enter_context(tc.tile_pool(name="rope", bufs=2))
        attn = ctx.enter_context(tc.tile_pool(name="attn", bufs=4))
        opool = ctx.enter_context(tc.tile_pool(name="opool", bufs=2))

        # ---- persistent constants (DMAs emitted in priority order) -----
        # wq/wk split in halves (km 0-7 / 8-15) so the first projection
        # matmuls start as soon as ~0.5 MB has landed.
        wq_sb = [const.tile([128, KM * DOUT // 2], BF, name=f"wq_sb{i}")
                 for i in range(2)]
        wk_sb = [const.tile([128, KM * DOUT // 2], BF, name=f"wk_sb{i}")
                 for i in range(2)]
        wv_sb = const.tile([128, KM * DOUT], BF, name="wv_sb")
        ones_sb = const.tile([128, 128], BF, name="ones_sb")
        cos_sb = const.tile([DH, S], BF, name="cos_sb")
        sin_sb = const.tile([DH, S], BF, name="sin_sb")  # rows 64-127 = -sin
        mask_sb = const.tile([SKB, 4 * SQ], BF, name="mask_sb")
        wo_sb = const.tile([128, HPC * D], BF, name="wo_sb")   # [p, jj*2048+n]

        qT = [qkv.tile([128, S], BF, tag=f"qT{j}", name=f"qT{j}") for j in range(HPC)]
        kT = [qkv.tile([128, S], BF, tag=f"kT{j}", name=f"kT{j}") for j in range(HPC)]
        # vsb: [seq-block u][j*128+dh] packed, both heads interleaved
        vsb = qkv.tile([128, (S // 128) * DOUT], BF, tag="vsb", name="vsb")

        with tc.tile_pool(name="psm", bufs=1, space="PSUM") as psm:
            # PSUM banks: pqk 2 (projections + deferred out-proj pf),
            # pscr 2x2 (attention score pairs), po 1, pd 1 = 8 banks.

            def emit_outproj_block(prev, mb, tags=("pqk",)):
                """Out-proj m-block mb (0..3) of the previous query tile.
                8 matmuls + 4 evacuations + 1 row DMA; interleaved into the
                attention phase to fill exp-paced PE gaps. prev carries the
                per-tile oT tiles so there is no false dependency on this
                tile's normalize writes."""
                if prev is None:
                    return
                pb, pt, poT = prev
                m = 4 * pt + mb
                ob = opool.tile([128, D], BF, tag="ob", bufs=2, name="ob")
                for n in range(D // 512):
                    pf = psm.tile([128, 512], F32, tag=tags[n % len(tags)],
                                  bufs=2, name="pf")
                    for jj in range(HPC):
                        nc.tensor.matmul(
                            pf[:], poT[jj][:, mb * 128:(mb + 1) * 128],
                            wo_sb[:, jj * D + n * 512: jj * D + (n + 1) * 512],
                            start=jj == 0, stop=jj == HPC - 1)
                    # late tiles: ScalarE is exp-saturated, keep evacs on DVE
                    if pt >= 3 or (m + n) % 2 == 0:
                        nc.vector.tensor_copy(ob[:, n * 512:(n + 1) * 512], pf[:])
                    else:
                        nc.scalar.copy(ob[:, n * 512:(n + 1) * 512], pf[:])
                nc.sync.dma_start(
                    outp[pb * S + m * 128: pb * S + (m + 1) * 128, :], ob[:])

            prev = None
            for b in range(B):
                for t in range(NSQ):
                    s0 = t * SQ
                    bt = b * NSQ + t
                    # --- x tile: 4 quarter tiles, contiguous DMAs -------
                    # quarter q holds contraction blocks km = 4q..4q+3.
                    xbt = [xpool.tile([128, XTILE // 2], BF, tag="xb", bufs=8,
                                      name=f"xbt{qq}") for qq in range(4)]
                    xsrc = [xP[:, ((qq // 2) * B * NSQ + bt) * XTILE
                               + (qq % 2) * (XTILE // 2):
                               ((qq // 2) * B * NSQ + bt) * XTILE
                               + (qq % 2 + 1) * (XTILE // 2)]
                            for qq in range(4)]
                    if bt == 0:
                        # x quarters on the sync HW queue; weights/tables on
                        # the scalar HW queue so the transfers run in
                        # parallel and the first matmul starts ~9us in.
                        for qq in range(4):
                            nc.sync.dma_start(xbt[qq][:], xsrc[qq])
                        nc.scalar.dma_start(wq_sb[0][:], wq[:, 0:KM * DOUT // 2])
                        nc.scalar.dma_start(wq_sb[1][:], wq[:, KM * DOUT // 2:])
                        nc.scalar.dma_start(wk_sb[0][:], wk[:, 0:KM * DOUT // 2])
                        nc.scalar.dma_start(wk_sb[1][:], wk[:, KM * DOUT // 2:])
                        nc.scalar.dma_start(wv_sb[:], wv[:])
                        nc.scalar.dma_start(cos_sb[:], cosf[:])
                        nc.scalar.dma_start(sin_sb[:], sins[:])
                        nc.scalar.dma_start(mask_sb[:], masks[:])
                        nc.scalar.dma_start(wo_sb[:], wo[:])
                        nc.scalar.dma_start(ones_sb[:], ones[:])
                    else:
                        for qq in range(4):
                            nc.sync.dma_start(xbt[qq][:], xsrc[qq])

                    def xsl(km, c0, c1):
                        return xbt[km // 4][:, (km % 4) * SQ + c0:
                                            (km % 4) * SQ + c1]

                    # --- q/k projections + RoPE, head by head -----------
                    for j in range(HPC):
                        for w_sb, dstt in ((wq_sb, qT[j]), (wk_sb, kT[j])):
                            pp = psm.tile([128, SQ], F32, tag="pqk", bufs=2,
                                          name="pp")
                            for km in range(KM):
                                nc.tensor.matmul(
                                    pp[:],
                                    w_sb[km // 8][
                                        :, (km % 8) * DOUT + j * DH:
                                        (km % 8) * DOUT + (j + 1) * DH],
                                    xsl(km, 0, SQ),
                                    start=km == 0, stop=km == KM - 1)
                            ppb = rope.tile([128, SQ], BF, tag="ppb", bufs=3,
                                            name="ppb")
                            nc.scalar.copy(ppb[:], pp[:])
                            rt = rope.tile([128, SQ], BF, tag="rot", bufs=2,
                                           name="rt")
                            # sin_sb rows 0:64 = +sin, rows 64:128 = -sin so
                            # both SBUF inputs share a base partition.
                            nc.vector.tensor_mul(
                                rt[0:HALF, :], ppb[HALF:128, :],
                                sin_sb[HALF:128, s0:s0 + SQ])
                            nc.vector.tensor_mul(
                                rt[HALF:128, :], ppb[0:HALF, :],
                                sin_sb[0:HALF, s0:s0 + SQ])
                            m1 = rope.tile([128, SQ], BF, tag="m1", bufs=2,
                                           name="m1")
                            nc.vector.tensor_mul(m1[:], ppb[:],
                                                 cos_sb[:, s0:s0 + SQ])
                            nc.vector.tensor_add(dstt[:, s0:s0 + SQ],
                                                 m1[:], rt[:])

                    # --- V projection directly in [seq, dh] layout ------
                    # stationary = x tile slice, moving = wv -> out rows are
                    # sequence positions; no transpose needed. Two seq
                    # sub-blocks share one PSUM bank (two matmul groups).
                    for sbp in range(2):
                        pv = psm.tile([128, 2 * DOUT], F32, tag="pqk", bufs=2,
                                      name="pv")
                        for sh in range(2):
                            sb = 2 * sbp + sh
                            for km in range(KM):
                                nc.tensor.matmul(
                                    pv[:, sh * DOUT:(sh + 1) * DOUT],
                                    xsl(km, sb * 128, (sb + 1) * 128),
                                    wv_sb[:, km * DOUT:(km + 1) * DOUT],
                                    start=km == 0, stop=km == KM - 1,
                                    skip_group_check=True)
                        u = 4 * t + 2 * sbp
                        nc.scalar.copy(
                            vsb[:, u * DOUT:(u + 2) * DOUT], pv[:])

                    # --- causal attention for this query tile -----------
                    # Out-proj blocks mb0/mb1 fill the j=0 exp-paced gaps,
                    # mb2/mb3 fill j=1. oT is per-tile so the deferred
                    # out-proj never aliases this tile's normalize writes.
                    oTt = []
                    for j in range(HPC):
                        nblk = 4 * t + 4
                        npair = nblk // 2
                        po = psm.tile([128, SQ], F32, tag="po", name="po")
                        pd = psm.tile([128, SQ], F32, tag="pd", name="pd")
                        prev_et = None
                        qs2s = []
                        for p in range(npair):
                            pscr = psm.tile([128, 2 * SQ], F32, tag="pscr",
                                            bufs=2, name="pscr")
                            diag = 2 * p >= 4 * t
                            for h in range(2):
                                u = 2 * p + h
                                off = (u - 4 * t) * SKB if (diag and bt) else 0
                                nc.tensor.matmul(
                                    pscr[:, h * SQ + off:(h + 1) * SQ],
                                    kT[j][:, u * SKB:(u + 1) * SKB],
                                    qT[j][:, s0 + off:s0 + SQ],
                                    start=True, stop=True,
                                    skip_group_check=True)
                            et = attn.tile([128, 2 * SQ], BF, tag="et", bufs=6,
                                           name="et")
                            if diag and bt and 2 * p - 4 * t == 2:
                                # steep diagonal pair: exp only the live
                                # regions [256:512] and [896:1024]
                                nc.scalar.activation(
                                    et[:, 256:512], pscr[:, 256:512],
                                    mybir.ActivationFunctionType.Exp,
                                    scale=SCALE)
                                nc.scalar.activation(
                                    et[:, 896:1024], pscr[:, 896:1024],
                                    mybir.ActivationFunctionType.Exp,
                                    scale=SCALE)
                            else:
                                nc.scalar.activation(
                                    et[:], pscr[:],
                                    mybir.ActivationFunctionType.Exp,
                                    scale=SCALE)
                            if diag:  # mask also zeroes any stale region
                                r = 2 * p - 4 * t   # 0 or 2
                                nc.vector.tensor_mul(
                                    et[:], et[:],
                                    mask_sb[:, r * SQ:(r + 2) * SQ])
                            for h in range(2):
                                u = 2 * p + h
                                off = (u - 4 * t) * SKB if diag else 0
                                nc.tensor.matmul(
                                    po[:, off:SQ],
                                    vsb[:, u * DOUT + j * DH:
                                        u * DOUT + (j + 1) * DH],
                                    et[:, h * SQ + off:(h + 1) * SQ],
                                    start=u == 0, stop=u == nblk - 1,
                                    skip_group_check=True)
                            if p % 2 == 1:
                                qs = attn.tile([128, 2 * SQ], BF, tag="qs",
                                               bufs=3, name="qs")
                                nc.vector.tensor_add(qs[:], prev_et[:], et[:])
                                qs2 = attn.tile([128, SQ], BF, tag="qs2",
                                                bufs=8, name="qs2")
                                nc.vector.tensor_add(
                                    qs2[:], qs[:, 0:SQ], qs[:, SQ:2 * SQ])
                                qs2s.append(qs2)
                            prev_et = et
                            # fill exp-paced gaps with deferred out-proj
                            if p in (0, 1) and t > 0:
                                emit_outproj_block(prev, 2 * j + p)
                        # pre-sum qs2 pairs on DVE to halve the ones-matmuls
                        dsum = []
                        for qi in range(0, len(qs2s) - 1, 2):
                            q4 = attn.tile([128, SQ], BF, tag="q4", bufs=4,
                                           name="q4")
                            nc.vector.tensor_add(
                                q4[:], qs2s[qi][:], qs2s[qi + 1][:])
                            dsum.append(q4)
                        if len(qs2s) % 2:
                            dsum.append(qs2s[-1])
                        for qi, q2 in enumerate(dsum):
                            nc.tensor.matmul(
                                pd[:], ones_sb[:], q2[:],
                                start=qi == 0, stop=qi == len(dsum) - 1)
                        if t == 0:
                            emit_outproj_block(prev, 2 * j)
                        rec = attn.tile([128, SQ], F32, tag="rec", bufs=2,
                                        name="rec")
                        nc.vector.reciprocal_approx_fast(rec[:], pd[:])
                        if t == 0:
                            emit_outproj_block(prev, 2 * j + 1)
                        ot = attn.tile([128, SQ], BF, tag=f"oT{j}", bufs=3,
                                       name="ot")
                        nc.vector.tensor_mul(ot[:], po[:], rec[:])
                        oTt.append(ot)
                    prev = (b, t, oTt)
            # final tile's out-proj: alternate PSUM tags for deeper pipeline
            for mb in range(4):
                emit_outproj_block(prev, mb, tags=("pqk", "pscr"))

    nc.compile()
    return nc


def _host_inputs(x, wq, wk, wv, wo, cos, sin):
    bf16 = ml_dtypes.bfloat16
    # xP[p, hh, bt, a, n] = x[b, t*512+n, hh*1024 + a*128 + p]
    xb = np.ascontiguousarray(
        x.reshape(B * S, D).T).astype(bf16)           # [D, B*S]
    xP = np.ascontiguousarray(
        xb.reshape(2, 8, 128, B, NSQ, SQ)
        .transpose(2, 0, 3, 4, 1, 5).reshape(128, -1))

    def pack_w(w):  # [D, 256] -> [128, km*256+n]
        return np.ascontiguousarray(
            w.reshape(KM, 128, DOUT).transpose(1, 0, 2).reshape(128, -1)
        ).astype(bf16)

    cos = np.asarray(cos, dtype=np.float32)        # [S, 64]
    sin = np.asarray(sin, dtype=np.float32)
    cosf = np.ascontiguousarray(
        np.concatenate([cos, cos], axis=1).T).astype(bf16)   # [128, S]
    sinf = np.concatenate([sin, -sin], axis=1).T   # rows 64-127 negated
    sinf = np.ascontiguousarray(sinf).astype(bf16)

    i = np.arange(SKB)[:, None]
    jj = np.arange(SQ)[None, :]
    masks = np.concatenate(
        [(i + r * SKB <= jj) for r in range(4)], axis=1).astype(bf16)
    ones_h = np.ones((128, 128), dtype=bf16)

    in_maps = []
    for c in range(NC):
        lo = c * DOUT
        wop = np.ascontiguousarray(
            wo[lo:lo + DOUT, :].reshape(HPC, 128, D)
            .transpose(1, 0, 2).reshape(128, -1)).astype(bf16)
        in_maps.append({
            "xP": xP,
            "wq": pack_w(np.ascontiguousarray(wq[:, lo:lo + DOUT])),
            "wk": pack_w(np.ascontiguousarray(wk[:, lo:lo + DOUT])),
            "wv": pack_w(np.ascontiguousarray(wv[:, lo:lo + DOUT])),
            "wo": wop,
            "cosf": cosf,
            "sins": sinf,
            "masks": masks,
            "ones": ones_h,
        })
    return in_maps


def kernel(x, wq, wk, wv, wo, cos, sin, _trace=False, _tmpdir=None):
    if "nc" not in _CACHED:
        _CACHED["nc"] = _build()
    nc = _CACHED["nc"]
    in_maps = _host_inputs(
        np.asarray(x, dtype=np.float32), np.asarray(wq, dtype=np.float32),
        np.asarray(wk, dtype=np.float32), np.asarray(wv, dtype=np.float32),
        np.asarray(wo, dtype=np.float32), cos, sin)
    res = bass_utils.run_bass_kernel_spmd(
        nc, in_maps, core_ids=list(range(NC)), trace=_trace, tmpdir=_tmpdir)
    acc = np.zeros((B * S, D), dtype=np.float32)
    for c in range(NC):
        acc += res.results[c]["outp"].astype(np.float32)
    out = acc.reshape(B, S, D)
    if _trace:
        _CACHED["last_results"] = res
    return out


# revision 26
# speedup vs baseline: 1.2031x; 1.0446x over previous
"""Dense causal transformer attention block on 8 Trainium2 NeuronCores.

Problem: out = CausalAttention(RoPE(x@wq, x@wk), x@wv) @ wo
  x [2, 4096, 2048], 16 heads x 128 dim, fp32 I/O.

Sharding: tensor-parallel over heads. Core c owns heads {2c, 2c+1}:
  - computes qT/kT ([head_dim, seq] layout) for its heads from the
    host-packed xP (all DMAs are contiguous-row), RoPE applied on-chip in
    bf16 (one ScalarE PSUM->SBUF copy, then 2x-mode DVE ops),
  - V is projected directly in [seq, head_dim] layout by using the x tile
    as the matmul stationary operand (no PE transposes needed),
  - runs causal attention in transposed form (scoresT = k @ qT so the
    softmax weights come out as the moving operand of the A@V matmul),
    with partial-width A@V matmuls on the diagonal band,
  - denominators via an all-ones [128,128] stationary matmul on
    DVE-pre-summed exp tiles; reciprocal via the fast custom DVE op,
  - output projection for query tile t is deferred into tile t+1's
    attention phase (interleaved m-blocks) so the softmax-normalize chain
    never stalls the in-order PE queue.
Host sums the 8 partial outputs (the wo row-parallel all-reduce).

Compute dtype bf16 (PE 1 col/cycle), accumulation fp32 in PSUM.
"""
import sys

for _p in ("/opt/trn_rl_repo",):
    if _p not in sys.path:
        sys.path.insert(0, _p)

import numpy as np
import ml_dtypes
from contextlib import ExitStack

import concourse.bass as bass
import concourse.tile as tile
from concourse import bacc, mybir
from concourse import bass_utils

B, S, D = 2, 4096, 2048
H, DH = 16, 128
HALF = DH // 2
NC = 8
HPC = H // NC          # heads per core = 2
DOUT = HPC * DH        # 256 local proj width
ROPE_BASE = 10000.0
SCALE = 1.0 / float(np.sqrt(DH))
SQ = 512               # query tile (free dim of scoresT)
SKB = 128              # key block (partitions of scoresT)
KM = D // 128          # 16 contraction blocks
NSQ = S // SQ          # 8 query tiles per batch
BF = mybir.dt.bfloat16
F32 = mybir.dt.float32

_CACHED = {}


def _build():
    nc = bacc.Bacc("TRN2", target_bir_lowering=False, debug=False, num_devices=NC)

    # xP: [128, (hh, b*8+t, a*512+n)] so each (b,t,hh) x-tile DMA is a
    # contiguous [128, 4096] read. Weights pre-packed the same way.
    xP = nc.dram_tensor("xP", [128, 2 * B * NSQ * 8 * SQ], BF,
                        kind="ExternalInput").ap()
    wq = nc.dram_tensor("wq", [128, KM * DOUT], BF, kind="ExternalInput").ap()
    wk = nc.dram_tensor("wk", [128, KM * DOUT], BF, kind="ExternalInput").ap()
    wv = nc.dram_tensor("wv", [128, KM * DOUT], BF, kind="ExternalInput").ap()
    wo = nc.dram_tensor("wo", [128, HPC * D], BF, kind="ExternalInput").ap()
    cosf = nc.dram_tensor("cosf", [DH, S], BF, kind="ExternalInput").ap()
    sins = nc.dram_tensor("sins", [DH, S], BF, kind="ExternalInput").ap()
    masks = nc.dram_tensor("masks", [SKB, 4 * SQ], BF, kind="ExternalInput").ap()
    ones = nc.dram_tensor("ones", [128, 128], BF, kind="ExternalInput").ap()
    outp = nc.dram_tensor("outp", [B * S, D], BF, kind="ExternalOutput").ap()

    XTILE = 8 * SQ     # 4096 cols per (b,t,hh) x tile

    with tile.TileContext(nc) as tc, ExitStack() as ctx:
        const = ctx.enter_context(tc.tile_pool(name="const", bufs=1))
        xpool = ctx.enter_context(tc.tile_pool(name="xpool", bufs=1))
        qkv = ctx.enter_context(tc.tile_pool(name="qkv", bufs=1))
        rope = ctx.enter_context(tc.tile_pool(name="rope", bufs=2))
        attn = ctx.enter_context(tc.tile_pool(name="attn", bufs=4))
        opool = ctx.enter_context(tc.tile_pool(name="opool", bufs=2))

        # ---- persistent constants (DMAs emitted in priority order) -----
        # wq/wk split in halves (km 0-7 / 8-15) so the first projection
        # matmuls start as soon as ~0.5 MB has landed.
        wq_sb = [const.tile([128, KM * DOUT // 2], BF, name=f"wq_sb{i}")
                 for i in range(2)]
        wk_sb = [const.tile([128, KM * DOUT // 2], BF, name=f"wk_sb{i}")
                 for i in range(2)]
        wv_sb = const.tile([128, KM * DOUT], BF, name="wv_sb")
        ones_sb = const.tile([128, 128], BF, name="ones_sb")
        cos_sb = const.tile([DH, S], BF, name="cos_sb")
        sin_sb = const.tile([DH, S], BF, name="sin_sb")  # rows 64-127 = -sin
        mask_sb = const.tile([SKB, 4 * SQ], BF, name="mask_sb")
        wo_sb = const.tile([128, HPC * D], BF, name="wo_sb")   # [p, jj*2048+n]

        qT = [qkv.tile([128, S], BF, tag=f"qT{j}", name=f"qT{j}") for j in range(HPC)]
        kT = [qkv.tile([128, S], BF, tag=f"kT{j}", name=f"kT{j}") for j in range(HPC)]
        # vsb: [seq-block u][j*128+dh] packed, both heads interleaved
        vsb = qkv.tile([128, (S // 128) * DOUT], BF, tag="vsb", name="vsb")

        with tc.tile_pool(name="psm", bufs=1, space="PSUM") as psm:
            # PSUM banks: pqk 2 (projections + deferred out-proj pf),
            # pscr 2x2 (attention score pairs), po 1, pd 1 = 8 banks.

            def emit_outproj_block(prev, mb, tags=("pqk",)):
                """Out-proj m-block mb (0..3) of the previous query tile.
                8 matmuls + 4 evacuations + 1 row DMA; interleaved into the
                attention phase to fill exp-paced PE gaps. prev carries the
                per-tile oT tiles so there is no false dependency on this
                tile's normalize writes."""
                if prev is None:
                    return
                pb, pt, poT = prev
                m = 4 * pt + mb
                ob = opool.tile([128, D], BF, tag="ob", bufs=2, name="ob")
                for n in range(D // 512):
                    pf = psm.tile([128, 512], F32, tag=tags[n % len(tags)],
                                  bufs=2, name="pf")
                    for jj in range(HPC):
                        nc.tensor.matmul(
                            pf[:], poT[jj][:, mb * 128:(mb + 1) * 128],
                            wo_sb[:, jj * D + n * 512: jj * D + (n + 1) * 512],
                            start=jj == 0, stop=jj == HPC - 1)
                    # late tiles: ScalarE is exp-saturated, keep evacs on DVE
                    if pt >= 3 or (m + n) % 2 == 0:
                        nc.vector.tensor_copy(ob[:, n * 512:(n + 1) * 512], pf[:])
                    else:
                        nc.scalar.copy(ob[:, n * 512:(n + 1) * 512], pf[:])
                nc.sync.dma_start(
                    outp[pb * S + m * 128: pb * S + (m + 1) * 128, :], ob[:])

            prev = None
            for b in range(B):
                for t in range(NSQ):
                    s0 = t * SQ
                    bt = b * NSQ + t
                    # --- x tile: 4 quarter tiles, contiguous DMAs -------
                    # quarter q holds contraction blocks km = 4q..4q+3.
                    xbt = [xpool.tile([128, XTILE // 2], BF, tag="xb", bufs=8,
                                      name=f"xbt{qq}") for qq in range(4)]
                    xsrc = [xP[:, ((qq // 2) * B * NSQ + bt) * XTILE
                               + (qq % 2) * (XTILE // 2):
                               ((qq // 2) * B * NSQ + bt) * XTILE
                               + (qq % 2 + 1) * (XTILE // 2)]
                            for qq in range(4)]
                    if bt == 0:
                        # x quarters on the sync HW queue; weights/tables on
                        # the scalar HW queue so the transfers run in
                        # parallel and the first matmul starts ~9us in.
                        for qq in range(4):
                            nc.sync.dma_start(xbt[qq][:], xsrc[qq])
                        nc.scalar.dma_start(wq_sb[0][:], wq[:, 0:KM * DOUT // 2])
                        nc.scalar.dma_start(wq_sb[1][:], wq[:, KM * DOUT // 2:])
                        nc.scalar.dma_start(wk_sb[0][:], wk[:, 0:KM * DOUT // 2])
                        nc.scalar.dma_start(wk_sb[1][:], wk[:, KM * DOUT // 2:])
                        nc.scalar.dma_start(wv_sb[:], wv[:])
                        nc.scalar.dma_start(cos_sb[:], cosf[:])
                        nc.scalar.dma_start(sin_sb[:], sins[:])
                        nc.scalar.dma_start(mask_sb[:], masks[:])
                        nc.scalar.dma_start(wo_sb[:], wo[:])
                        nc.scalar.dma_start(ones_sb[:], ones[:])
                    else:
                        for qq in range(4):
                            nc.sync.dma_start(xbt[qq][:], xsrc[qq])

                    def xsl(km, c0, c1):
                        return xbt[km // 4][:, (km % 4) * SQ + c0:
                                            (km % 4) * SQ + c1]

                    # --- q/k projections + RoPE, head by head -----------
                    for j in range(HPC):
                        for w_sb, dstt in ((wq_sb, qT[j]), (wk_sb, kT[j])):
                            pp = psm.tile([128, SQ], F32, tag="pqk", bufs=2,
                                          name="pp")
                            for km in range(KM):
                                nc.tensor.matmul(
                                    pp[:],
                                    w_sb[km // 8][
                                        :, (km % 8) * DOUT + j * DH:
                                        (km % 8) * DOUT + (j + 1) * DH],
                                    xsl(km, 0, SQ),
                                    start=km == 0, stop=km == KM - 1)
                            ppb = rope.tile([128, SQ], BF, tag="ppb", bufs=3,
                                            name="ppb")
                            nc.scalar.copy(ppb[:], pp[:])
                            rt = rope.tile([128, SQ], BF, tag="rot", bufs=2,
                                           name="rt")
                            # sin_sb rows 0:64 = +sin, rows 64:128 = -sin so
                            # both SBUF inputs share a base partition.
                            nc.vector.tensor_mul(
                                rt[0:HALF, :], ppb[HALF:128, :],
                                sin_sb[HALF:128, s0:s0 + SQ])
                            nc.vector.tensor_mul(
                                rt[HALF:128, :], ppb[0:HALF, :],
                                sin_sb[0:HALF, s0:s0 + SQ])
                            m1 = rope.tile([128, SQ], BF, tag="m1", bufs=2,
                                           name="m1")
                            nc.vector.tensor_mul(m1[:], ppb[:],
                                                 cos_sb[:, s0:s0 + SQ])
                            nc.vector.tensor_add(dstt[:, s0:s0 + SQ],
                                                 m1[:], rt[:])

                    # --- V projection directly in [seq, dh] layout ------
                    # stationary = x tile slice, moving = wv -> out rows are
                    # sequence positions; no transpose needed. Two seq
                    # sub-blocks share one PSUM bank (two matmul groups).
                    for sbp in range(2):
                        pv = psm.tile([128, 2 * DOUT], F32, tag="pqk", bufs=2,
                                      name="pv")
                        for sh in range(2):
                            sb = 2 * sbp + sh
                            for km in range(KM):
                                nc.tensor.matmul(
                                    pv[:, sh * DOUT:(sh + 1) * DOUT],
                                    xsl(km, sb * 128, (sb + 1) * 128),
                                    wv_sb[:, km * DOUT:(km + 1) * DOUT],
                                    start=km == 0, stop=km == KM - 1,
                                    skip_group_check=True)
                        u = 4 * t + 2 * sbp
                        nc.scalar.copy(
                            vsb[:, u * DOUT:(u + 2) * DOUT], pv[:])

                    # --- causal attention for this query tile -----------
                    # Out-proj blocks mb0/mb1 fill the j=0 exp-paced gaps,
                    # mb2/mb3 fill j=1. oT is per-tile so the deferred
                    # out-proj never aliases this tile's normalize writes.
                    oTt = []
                    for j in range(HPC):
                        nblk = 4 * t + 4
                        npair = nblk // 2
                        po = psm.tile([128, SQ], F32, tag="po", name="po")
                        pd = psm.tile([128, SQ], F32, tag="pd", name="pd")
                        prev_et = None
                        qs2s = []
                        for p in range(npair):
                            pscr = psm.tile([128, 2 * SQ], F32, tag="pscr",
                                            bufs=2, name="pscr")
                            diag = 2 * p >= 4 * t
                            for h in range(2):
                                u = 2 * p + h
                                off = (u - 4 * t) * SKB if (diag and bt) else 0
                                nc.tensor.matmul(
                                    pscr[:, h * SQ + off:(h + 1) * SQ],
                                    kT[j][:, u * SKB:(u + 1) * SKB],
                                    qT[j][:, s0 + off:s0 + SQ],
                                    start=True, stop=True,
                                    skip_group_check=True)
                            et = attn.tile([128, 2 * SQ], BF, tag="et", bufs=6,
                                           name="et")
                            if diag and bt and 2 * p - 4 * t == 2:
                                # steep diagonal pair: exp only the live
                                # regions [256:512] and [896:1024]
                                nc.scalar.activation(
                                    et[:, 256:512], pscr[:, 256:512],
                                    mybir.ActivationFunctionType.Exp,
                                    scale=SCALE)
                                nc.scalar.activation(
                                    et[:, 896:1024], pscr[:, 896:1024],
                                    mybir.ActivationFunctionType.Exp,
                                    scale=SCALE)
                            else:
                                nc.scalar.activation(
                                    et[:], pscr[:],
                                    mybir.ActivationFunctionType.Exp,
                                    scale=SCALE)
                            if diag:  # mask also zeroes any stale region
                                r = 2 * p - 4 * t   # 0 or 2
                                nc.vector.tensor_mul(
                                    et[:], et[:],
                                    mask_sb[:, r * SQ:(r + 2) * SQ])
                            for h in range(2):
                                u = 2 * p + h
                                off = (u - 4 * t) * SKB if diag else 0
                                nc.tensor.matmul(
                                    po[:, off:SQ],
                                    vsb[:, u * DOUT + j * DH:
                                        u * DOUT + (j + 1) * DH],
                                    et[:, h * SQ + off:(h + 1) * SQ],
                                    start=u == 0, stop=u == nblk - 1,
                                    skip_group_check=True)
                            if p % 2 == 1:
                                qs = attn.tile([128, 2 * SQ], BF, tag="qs",
                                               bufs=3, name="qs")
                                nc.vector.tensor_add(qs[:], prev_et[:], et[:])
                                qs2 = attn.tile([128, SQ], BF, tag="qs2",
                                                bufs=8, name="qs2")
                                nc.vector.tensor_add(
                                    qs2[:], qs[:, 0:SQ], qs[:, SQ:2 * SQ])
                                qs2s.append(qs2)
                            prev_et = et
                            # fill exp-paced gaps with deferred out-proj
                            if t > 0 and ((j, p) == (0, 1) or (j, p) == (1, 0)):
                                emit_outproj_block(prev, 0 if j == 0 else 3)
                        # pre-sum qs2 pairs on DVE to halve the ones-matmuls
                        dsum = []
                        for qi in range(0, len(qs2s) - 1, 2):
                            q4 = attn.tile([128, SQ], BF, tag="q4", bufs=4,
                                           name="q4")
                            nc.vector.tensor_add(
                                q4[:], qs2s[qi][:], qs2s[qi + 1][:])
                            dsum.append(q4)
                        if len(qs2s) % 2:
                            dsum.append(qs2s[-1])
                        for qi, q2 in enumerate(dsum):
                            nc.tensor.matmul(
                                pd[:], ones_sb[:], q2[:],
                                start=qi == 0, stop=qi == len(dsum) - 1)
                        if t == 0:
                            emit_outproj_block(prev, 2 * j)
                        elif j == 0:
                            emit_outproj_block(prev, 1)  # cover j0 end chain
                        rec = attn.tile([128, SQ], F32, tag="rec", bufs=2,
                                        name="rec")
                        nc.vector.reciprocal_approx_fast(rec[:], pd[:])
                        if t == 0:
                            emit_outproj_block(prev, 2 * j + 1)
                        elif j == 0:
                            emit_outproj_block(prev, 2)
                        ot = attn.tile([128, SQ], BF, tag=f"oT{j}", bufs=3,
                                       name="ot")
                        nc.vector.tensor_mul(ot[:], po[:], rec[:])
                        oTt.append(ot)
                    prev = (b, t, oTt)
            # final tile's out-proj: alternate PSUM tags for deeper pipeline
            for mb in range(4):
                emit_outproj_block(prev, mb, tags=("pqk", "pscr"))

    nc.compile()
    return nc


def _host_inputs(x, wq, wk, wv, wo, cos, sin):
    bf16 = ml_dtypes.bfloat16
    # xP[p, hh, bt, a, n] = x[b, t*512+n, hh*1024 + a*128 + p]
    xb = np.ascontiguousarray(
        x.reshape(B * S, D).T).astype(bf16)           # [D, B*S]
    xP = np.ascontiguousarray(
        xb.reshape(2, 8, 128, B, NSQ, SQ)
        .transpose(2, 0, 3, 4, 1, 5).reshape(128, -1))

    def pack_w(w):  # [D, 256] -> [128, km*256+n]
        return np.ascontiguousarray(
            w.reshape(KM, 128, DOUT).transpose(1, 0, 2).reshape(128, -1)
        ).astype(bf16)

    cos = np.asarray(cos, dtype=np.float32)        # [S, 64]
    sin = np.asarray(sin, dtype=np.float32)
    cosf = np.ascontiguousarray(
        np.concatenate([cos, cos], axis=1).T).astype(bf16)   # [128, S]
    sinf = np.concatenate([sin, -sin], axis=1).T   # rows 64-127 negated
    sinf = np.ascontiguousarray(sinf).astype(bf16)

    i = np.arange(SKB)[:, None]
    jj = np.arange(SQ)[None, :]
    masks = np.concatenate(
        [(i + r * SKB <= jj) for r in range(4)], axis=1).astype(bf16)
    ones_h = np.ones((128, 128), dtype=bf16)

    in_maps = []
    for c in range(NC):
        lo = c * DOUT
        wop = np.ascontiguousarray(
            wo[lo:lo + DOUT, :].reshape(HPC, 128, D)
            .transpose(1, 0, 2).reshape(128, -1)).astype(bf16)
        in_maps.append({
            "xP": xP,
            "wq": pack_w(np.ascontiguousarray(wq[:, lo:lo + DOUT])),
            "wk": pack_w(np.ascontiguousarray(wk[:, lo:lo + DOUT])),
            "wv": pack_w(np.ascontiguousarray(wv[:, lo:lo + DOUT])),
            "wo": wop,
            "cosf": cosf,
            "sins": sinf,
            "masks": masks,
            "ones": ones_h,
        })
    return in_maps


def kernel(x, wq, wk, wv, wo, cos, sin, _trace=False, _tmpdir=None):
    if "nc" not in _CACHED:
        _CACHED["nc"] = _build()
    nc = _CACHED["nc"]
    in_maps = _host_inputs(
        np.asarray(x, dtype=np.float32), np.asarray(wq, dtype=np.float32),
        np.asarray(wk, dtype=np.float32), np.asarray(wv, dtype=np.float32),
        np.asarray(wo, dtype=np.float32), cos, sin)
    res = bass_utils.run_bass_kernel_spmd(
        nc, in_maps, core_ids=list(range(NC)), trace=_trace, tmpdir=_tmpdir)
    acc = np.zeros((B * S, D), dtype=np.float32)
    for c in range(NC):
        acc += res.results[c]["outp"].astype(np.float32)
    out = acc.reshape(B, S, D)
    if _trace:
        _CACHED["last_results"] = res
    return out


# revision 38
# speedup vs baseline: 1.2066x; 1.0029x over previous
"""Dense causal transformer attention block on 8 Trainium2 NeuronCores.

Problem: out = CausalAttention(RoPE(x@wq, x@wk), x@wv) @ wo
  x [2, 4096, 2048], 16 heads x 128 dim, fp32 I/O.

Sharding: tensor-parallel over heads. Core c owns heads {2c, 2c+1}:
  - computes qT/kT ([head_dim, seq] layout) for its heads from the
    host-packed xP (all DMAs are contiguous-row), RoPE applied on-chip in
    bf16 (one ScalarE PSUM->SBUF copy, then 2x-mode DVE ops),
  - V is projected directly in [seq, head_dim] layout by using the x tile
    as the matmul stationary operand (no PE transposes needed),
  - runs causal attention in transposed form (scoresT = k @ qT so the
    softmax weights come out as the moving operand of the A@V matmul),
    with partial-width A@V matmuls on the diagonal band,
  - denominators via an all-ones [128,128] stationary matmul on
    DVE-pre-summed exp tiles; reciprocal via the fast custom DVE op,
  - output projection for query tile t is deferred into tile t+1's
    attention phase (interleaved m-blocks) so the softmax-normalize chain
    never stalls the in-order PE queue.
Host sums the 8 partial outputs (the wo row-parallel all-reduce).

Compute dtype bf16 (PE 1 col/cycle), accumulation fp32 in PSUM.
"""
import sys

for _p in ("/opt/trn_rl_repo",):
    if _p not in sys.path:
        sys.path.insert(0, _p)

import numpy as np
import ml_dtypes
from contextlib import ExitStack

import concourse.bass as bass
import concourse.tile as tile
from concourse import bacc, mybir
from concourse import bass_utils

B, S, D = 2, 4096, 2048
H, DH = 16, 128
HALF = DH // 2
NC = 8
HPC = H // NC          # heads per core = 2
DOUT = HPC * DH        # 256 local proj width
ROPE_BASE = 10000.0
SCALE = 1.0 / float(np.sqrt(DH))
SQ = 512               # query tile (free dim of scoresT)
SKB = 128              # key block (partitions of scoresT)
KM = D // 128          # 16 contraction blocks
NSQ = S // SQ          # 8 query tiles per batch
BF = mybir.dt.bfloat16
F32 = mybir.dt.float32

_CACHED = {}


def _build():
    nc = bacc.Bacc("TRN2", target_bir_lowering=False, debug=False, num_devices=NC)

    # xP: [128, (hh, b*8+t, a*512+n)] so each (b,t,hh) x-tile DMA is a
    # contiguous [128, 4096] read. Weights pre-packed the same way.
    xP = nc.dram_tensor("xP", [128, 2 * B * NSQ * 8 * SQ], BF,
                        kind="ExternalInput").ap()
    wq = nc.dram_tensor("wq", [128, KM * DOUT], BF, kind="ExternalInput").ap()
    wk = nc.dram_tensor("wk", [128, KM * DOUT], BF, kind="ExternalInput").ap()
    wv = nc.dram_tensor("wv", [128, KM * DOUT], BF, kind="ExternalInput").ap()
    wo = nc.dram_tensor("wo", [128, HPC * D], BF, kind="ExternalInput").ap()
    cosf = nc.dram_tensor("cosf", [DH, S], BF, kind="ExternalInput").ap()
    sins = nc.dram_tensor("sins", [DH, S], BF, kind="ExternalInput").ap()
    masks = nc.dram_tensor("masks", [SKB, 4 * SQ], BF, kind="ExternalInput").ap()
    ones = nc.dram_tensor("ones", [128, 128], BF, kind="ExternalInput").ap()
    outp = nc.dram_tensor("outp", [B * S, D], BF, kind="ExternalOutput").ap()

    XTILE = 8 * SQ     # 4096 cols per (b,t,hh) x tile

    with tile.TileContext(nc) as tc, ExitStack() as ctx:
        const = ctx.enter_context(tc.tile_pool(name="const", bufs=1))
        xpool = ctx.enter_context(tc.tile_pool(name="xpool", bufs=1))
        qkv = ctx.enter_context(tc.tile_pool(name="qkv", bufs=1))
        rope = ctx.enter_context(tc.tile_pool(name="rope", bufs=2))
        attn = ctx.enter_context(tc.tile_pool(name="attn", bufs=4))
        opool = ctx.enter_context(tc.tile_pool(name="opool", bufs=2))

        # ---- persistent constants (DMAs emitted in priority order) -----
        # wq/wk split in halves (km 0-7 / 8-15) so the first projection
        # matmuls start as soon as ~0.5 MB has landed.
        wq_sb = [const.tile([128, KM * DOUT // 2], BF, name=f"wq_sb{i}")
                 for i in range(2)]
        wk_sb = [const.tile([128, KM * DOUT // 2], BF, name=f"wk_sb{i}")
                 for i in range(2)]
        wv_sb = const.tile([128, KM * DOUT], BF, name="wv_sb")
        ones_sb = const.tile([128, 128], BF, name="ones_sb")
        cos_sb = const.tile([DH, S], BF, name="cos_sb")
        sin_sb = const.tile([DH, S], BF, name="sin_sb")  # rows 64-127 = -sin
        mask_sb = const.tile([SKB, 4 * SQ], BF, name="mask_sb")
        wo_sb = const.tile([128, HPC * D], BF, name="wo_sb")   # [p, jj*2048+n]

        qT = [qkv.tile([128, S], BF, tag=f"qT{j}", name=f"qT{j}") for j in range(HPC)]
        kT = [qkv.tile([128, S], BF, tag=f"kT{j}", name=f"kT{j}") for j in range(HPC)]
        # vsb is per-tile (4 seq-blocks x [j*128+dh]), allocated from a
        # rotating pool so deferred/interleaved V projections never create
        # false dependencies against attention reads of older tiles.

        with tc.tile_pool(name="psm", bufs=1, space="PSUM") as psm:
            # PSUM banks: pqk 2 (projections + deferred out-proj pf),
            # pscr 2x2 (attention score pairs), po 1, pd 1 = 8 banks.

            def emit_outproj_block(prev, mb, tags=("pqk",), split_evac=False):
                """Out-proj m-block mb (0..3) of the previous query tile.
                8 matmuls + 4 evacuations + 1 row DMA; interleaved into the
                attention phase to fill exp-paced PE gaps. prev carries the
                per-tile oT tiles so there is no false dependency on this
                tile's normalize writes."""
                if prev is None:
                    return
                pb, pt, poT = prev
                m = 4 * pt + mb
                ob = opool.tile([128, D], BF, tag="ob", bufs=2, name="ob")
                for n in range(D // 512):
                    pf = psm.tile([128, 512], F32, tag=tags[n % len(tags)],
                                  bufs=2, name="pf")
                    for jj in range(HPC):
                        nc.tensor.matmul(
                            pf[:], poT[jj][:, mb * 128:(mb + 1) * 128],
                            wo_sb[:, jj * D + n * 512: jj * D + (n + 1) * 512],
                            start=jj == 0, stop=jj == HPC - 1)
                    # late tiles: ScalarE is exp-saturated, keep evacs on DVE
                    if split_evac:
                        on_sce = (m + n) % 2 == 1
                    else:
                        on_sce = pt < 3 and (m + n) % 2 == 1
                    if on_sce:
                        nc.scalar.copy(ob[:, n * 512:(n + 1) * 512], pf[:])
                    else:
                        nc.vector.tensor_copy(ob[:, n * 512:(n + 1) * 512], pf[:])
                nc.sync.dma_start(
                    outp[pb * S + m * 128: pb * S + (m + 1) * 128, :], ob[:])

            prev = None
            xbts = {}
            vsb_tiles = {}
            NT = B * NSQ

            def emit_xdma(bt2):
                """x quarter tiles + contiguous DMAs for tile bt2."""
                if bt2 >= NT:
                    return
                xbt = [xpool.tile([128, XTILE // 2], BF, tag="xb", bufs=8,
                                  name=f"xbt{qq}") for qq in range(4)]
                xbts[bt2] = xbt
                for qq in range(4):
                    nc.sync.dma_start(
                        xbt[qq][:],
                        xP[:, ((qq // 2) * NT + bt2) * XTILE
                           + (qq % 2) * (XTILE // 2):
                           ((qq // 2) * NT + bt2) * XTILE
                           + (qq % 2 + 1) * (XTILE // 2)])

            def emit_vproj_half(bt2, sbp):
                """V projection for tile bt2 (one pair of seq sub-blocks),
                directly in [seq, dh] layout: stationary = x tile slice,
                moving = wv. Interleaved into the previous tile's attention
                phase as extra PE fill work."""
                if bt2 >= NT:
                    return
                if sbp == 0:
                    vsb_tiles[bt2] = attn.tile([128, 4 * DOUT], BF, tag="vsb",
                                               bufs=10, name="vtile")
                vtile = vsb_tiles[bt2]
                xb = xbts[bt2]
                pv = psm.tile([128, 2 * DOUT], F32, tag="pqk", bufs=2,
                              name="pv")
                for sh in range(2):
                    sb = 2 * sbp + sh
                    for km in range(KM):
                        nc.tensor.matmul(
                            pv[:, sh * DOUT:(sh + 1) * DOUT],
                            xb[km // 4][:, (km % 4) * SQ + sb * 128:
                                        (km % 4) * SQ + (sb + 1) * 128],
                            wv_sb[:, km * DOUT:(km + 1) * DOUT],
                            start=km == 0, stop=km == KM - 1,
                            skip_group_check=True)
                nc.scalar.copy(
                    vtile[:, 2 * sbp * DOUT:(2 * sbp + 2) * DOUT], pv[:])

            for b in range(B):
                for t in range(NSQ):
                    s0 = t * SQ
                    bt = b * NSQ + t
                    if bt == 0:
                        emit_xdma(0)
                        # wq on the scalar HW queue in parallel with x on
                        # the sync queue; remaining consts are emitted at
                        # later scalar-queue positions (below) so they do
                        # not steal bandwidth from the first-needed DMAs.
                        nc.scalar.dma_start(wq_sb[0][:],
                                            wq[:, 0:KM * DOUT // 2])
                        nc.scalar.dma_start(wq_sb[1][:],
                                            wq[:, KM * DOUT // 2:])
                        # cos/sin before any RoPE op is emitted (a later
                        # emission would invert the dependency)
                        nc.scalar.dma_start(cos_sb[:], cosf[:])
                        nc.scalar.dma_start(sin_sb[:], sins[:])
                        nc.scalar.dma_start(wk_sb[0][:],
                                            wk[:, 0:KM * DOUT // 2])
                        nc.scalar.dma_start(wk_sb[1][:],
                                            wk[:, KM * DOUT // 2:])
                        nc.scalar.dma_start(wv_sb[:], wv[:])
                        emit_xdma(1)
                    elif bt + 1 < NT:
                        emit_xdma(bt + 1)
                    xbt = xbts[bt]

                    def xsl(km, c0, c1):
                        return xbt[km // 4][:, (km % 4) * SQ + c0:
                                            (km % 4) * SQ + c1]

                    # --- q/k projections + RoPE, head by head -----------
                    pi = 0
                    for j in range(HPC):
                        for w_sb, dstt in ((wq_sb, qT[j]), (wk_sb, kT[j])):
                            pp = psm.tile([128, SQ], F32, tag="pqk", bufs=2,
                                          name="pp")
                            for km in range(KM):
                                nc.tensor.matmul(
                                    pp[:],
                                    w_sb[km // 8][
                                        :, (km % 8) * DOUT + j * DH:
                                        (km % 8) * DOUT + (j + 1) * DH],
                                    xsl(km, 0, SQ),
                                    start=km == 0, stop=km == KM - 1)
                            ppb = rope.tile([128, SQ], BF, tag="ppb", bufs=3,
                                            name="ppb")
                            nc.scalar.copy(ppb[:], pp[:])
                            rt = rope.tile([128, SQ], BF, tag="rot", bufs=2,
                                           name="rt")
                            # sin_sb rows 0:64 = +sin, rows 64:128 = -sin so
                            # both SBUF inputs share a base partition.
                            nc.vector.tensor_mul(
                                rt[0:HALF, :], ppb[HALF:128, :],
                                sin_sb[HALF:128, s0:s0 + SQ])
                            nc.vector.tensor_mul(
                                rt[HALF:128, :], ppb[0:HALF, :],
                                sin_sb[0:HALF, s0:s0 + SQ])
                            m1 = rope.tile([128, SQ], BF, tag="m1", bufs=2,
                                           name="m1")
                            nc.vector.tensor_mul(m1[:], ppb[:],
                                                 cos_sb[:, s0:s0 + SQ])
                            nc.vector.tensor_add(dstt[:, s0:s0 + SQ],
                                                 m1[:], rt[:])
                            if bt == 0:
                                # masks/wo/ones staggered late: their first
                                # consumers are emitted later than this
                                if pi == 2:
                                    nc.scalar.dma_start(mask_sb[:], masks[:])
                                elif pi == 3:
                                    nc.scalar.dma_start(wo_sb[:], wo[:])
                                    nc.scalar.dma_start(ones_sb[:], ones[:])
                                pi += 1

                    if bt == 0:
                        # bootstrap: tile 0's own V projection
                        emit_vproj_half(0, 0)
                        emit_vproj_half(0, 1)

                    # --- causal attention for this query tile -----------
                    # (also emits the NEXT tile's V projection as PE fill)
                    # Out-proj blocks mb0/mb1 fill the j=0 exp-paced gaps,
                    # mb2/mb3 fill j=1. oT is per-tile so the deferred
                    # out-proj never aliases this tile's normalize writes.
                    oTt = []
                    for j in range(HPC):
                        nblk = 4 * t + 4
                        npair = nblk // 2
                        po = psm.tile([128, SQ], F32, tag="po", name="po")
                        pd = psm.tile([128, SQ], F32, tag="pd", name="pd")
                        prev_et = None
                        qs2s = []
                        for p in range(npair):
                            pscr = psm.tile([128, 2 * SQ], F32, tag="pscr",
                                            bufs=2, name="pscr")
                            diag = 2 * p >= 4 * t
                            for h in range(2):
                                u = 2 * p + h
                                off = (u - 4 * t) * SKB if (diag and bt) else 0
                                nc.tensor.matmul(
                                    pscr[:, h * SQ + off:(h + 1) * SQ],
                                    kT[j][:, u * SKB:(u + 1) * SKB],
                                    qT[j][:, s0 + off:s0 + SQ],
                                    start=True, stop=True,
                                    skip_group_check=True)
                            et = attn.tile([128, 2 * SQ], BF, tag="et", bufs=6,
                                           name="et")
                            if diag and bt and 2 * p - 4 * t == 2:
                                # steep diagonal pair: exp only the live
                                # regions [256:512] and [896:1024]
                                nc.scalar.activation(
                                    et[:, 256:512], pscr[:, 256:512],
                                    mybir.ActivationFunctionType.Exp,
                                    scale=SCALE)
                                nc.scalar.activation(
                                    et[:, 896:1024], pscr[:, 896:1024],
                                    mybir.ActivationFunctionType.Exp,
                                    scale=SCALE)
                            else:
                                nc.scalar.activation(
                                    et[:], pscr[:],
                                    mybir.ActivationFunctionType.Exp,
                                    scale=SCALE)
                            if diag:  # mask also zeroes any stale region
                                r = 2 * p - 4 * t   # 0 or 2
                                nc.vector.tensor_mul(
                                    et[:], et[:],
                                    mask_sb[:, r * SQ:(r + 2) * SQ])
                            for h in range(2):
                                u = 2 * p + h
                                off = (u - 4 * t) * SKB if diag else 0
                                vt_ = vsb_tiles[b * NSQ + u // 4]
                                nc.tensor.matmul(
                                    po[:, off:SQ],
                                    vt_[:, (u % 4) * DOUT + j * DH:
                                        (u % 4) * DOUT + (j + 1) * DH],
                                    et[:, h * SQ + off:(h + 1) * SQ],
                                    start=u == 0, stop=u == nblk - 1,
                                    skip_group_check=True)
                            if p % 2 == 1:
                                qs = attn.tile([128, 2 * SQ], BF, tag="qs",
                                               bufs=3, name="qs")
                                nc.vector.tensor_add(qs[:], prev_et[:], et[:])
                                qs2 = attn.tile([128, SQ], BF, tag="qs2",
                                                bufs=8, name="qs2")
                                nc.vector.tensor_add(
                                    qs2[:], qs[:, 0:SQ], qs[:, SQ:2 * SQ])
                                qs2s.append(qs2)
                            prev_et = et
                            # fill exp-paced gaps: deferred out-proj blocks
                            # and the next tile's V projection
                            if t > 0 and ((j, p) == (0, 1) or (j, p) == (1, 0)):
                                emit_outproj_block(prev, 0 if j == 0 else 3)
                            if t > 0 and ((j, p) == (0, 2) or (j, p) == (1, 1)):
                                emit_vproj_half(bt + 1, 0 if j == 0 else 1)
                        # pre-sum qs2 pairs on DVE to halve the ones-matmuls
                        dsum = []
                        for qi in range(0, len(qs2s) - 1, 2):
                            q4 = attn.tile([128, SQ], BF, tag="q4", bufs=4,
                                           name="q4")
                            nc.vector.tensor_add(
                                q4[:], qs2s[qi][:], qs2s[qi + 1][:])
                            dsum.append(q4)
                        if len(qs2s) % 2:
                            dsum.append(qs2s[-1])
                        for qi, q2 in enumerate(dsum):
                            nc.tensor.matmul(
                                pd[:], ones_sb[:], q2[:],
                                start=qi == 0, stop=qi == len(dsum) - 1)
                        if t == 0:
                            emit_outproj_block(prev, 2 * j)
                        elif j == 0:
                            emit_outproj_block(prev, 1)  # cover j0 end chain
                        rec = attn.tile([128, SQ], F32, tag="rec", bufs=2,
                                        name="rec")
                        nc.vector.reciprocal_approx_fast(rec[:], pd[:])
                        if t == 0:
                            emit_outproj_block(prev, 2 * j + 1)
                            emit_vproj_half(bt + 1, j)
                        elif j == 0:
                            emit_outproj_block(prev, 2)
                        ot = attn.tile([128, SQ], BF, tag=f"oT{j}", bufs=3,
                                       name="ot")
                        nc.vector.tensor_mul(ot[:], po[:], rec[:])
                        oTt.append(ot)
                    prev = (b, t, oTt)
            # final tile's out-proj: alternate PSUM tags for deeper pipeline
            for mb in range(4):
                emit_outproj_block(prev, mb, tags=("pqk", "pscr"),
                                   split_evac=True)

    nc.compile()
    return nc


def _host_inputs(x, wq, wk, wv, wo, cos, sin):
    bf16 = ml_dtypes.bfloat16
    # xP[p, hh, bt, a, n] = x[b, t*512+n, hh*1024 + a*128 + p]
    xb = np.ascontiguousarray(
        x.reshape(B * S, D).T).astype(bf16)           # [D, B*S]
    xP = np.ascontiguousarray(
        xb.reshape(2, 8, 128, B, NSQ, SQ)
        .transpose(2, 0, 3, 4, 1, 5).reshape(128, -1))

    def pack_w(w):  # [D, 256] -> [128, km*256+n]
        return np.ascontiguousarray(
            w.reshape(KM, 128, DOUT).transpose(1, 0, 2).reshape(128, -1)
        ).astype(bf16)

    cos = np.asarray(cos, dtype=np.float32)        # [S, 64]
    sin = np.asarray(sin, dtype=np.float32)
    cosf = np.ascontiguousarray(
        np.concatenate([cos, cos], axis=1).T).astype(bf16)   # [128, S]
    sinf = np.concatenate([sin, -sin], axis=1).T   # rows 64-127 negated
    sinf = np.ascontiguousarray(sinf).astype(bf16)

    i = np.arange(SKB)[:, None]
    jj = np.arange(SQ)[None, :]
    masks = np.concatenate(
        [(i + r * SKB <= jj) for r in range(4)], axis=1).astype(bf16)
    ones_h = np.ones((128, 128), dtype=bf16)

    in_maps = []
    for c in range(NC):
        lo = c * DOUT
        wop = np.ascontiguousarray(
            wo[lo:lo + DOUT, :].reshape(HPC, 128, D)
            .transpose(1, 0, 2).reshape(128, -1)).astype(bf16)
        in_maps.append({
            "xP": xP,
            "wq": pack_w(np.ascontiguousarray(wq[:, lo:lo + DOUT])),
            "wk": pack_w(np.ascontiguousarray(wk[:, lo:lo + DOUT])),
            "wv": pack_w(np.ascontiguousarray(wv[:, lo:lo + DOUT])),
            "wo": wop,
            "cosf": cosf,
            "sins": sinf,
            "masks": masks,
            "ones": ones_h,
        })
    return in_maps


def kernel(x, wq, wk, wv, wo, cos, sin, _trace=False, _tmpdir=None):
    if "nc" not in _CACHED:
        _CACHED["nc"] = _build()
    nc = _CACHED["nc"]
    in_maps = _host_inputs(
        np.asarray(x, dtype=np.float32), np.asarray(wq, dtype=np.float32),
        np.asarray(wk, dtype=np.float32), np.asarray(wv, dtype=np.float32),
        np.asarray(wo, dtype=np.float32), cos, sin)
    res = bass_utils.run_bass_kernel_spmd(
        nc, in_maps, core_ids=list(range(NC)), trace=_trace, tmpdir=_tmpdir)
    acc = np.zeros((B * S, D), dtype=np.float32)
    for c in range(NC):
        acc += res.results[c]["outp"].astype(np.float32)
    out = acc.reshape(B, S, D)
    if _trace:
        _CACHED["last_results"] = res
    return out
